# revision 2
# baseline (speedup 1.0000x reference)
"""CharLSTM Trainium2 kernel: 8-way tensor-parallel over the 4H gate dim,
layer-pipelined scan (layer2 lags layer1 by one step), per-tick cross-core
h-exchange via ncfw AllGather.

Math (matches reference):
  x = embed[idx]; 2-layer LSTM over T; out = hs @ W_out.
  Layer-1 input projection is folded into a gather: G1[b,t] = E1[idx[b,t]]
  with E1 = embed @ Wx[0] + b[0], realized on-device as a one-hot matmul
  (K = V = 128) so the whole scan stays on-chip.

Per-core slice: gate columns [i_j | f_j | o_j | g_j], each 128 wide
(H-slice j of each gate), 512 columns total. Per tick (= one timestep of
layer 1 plus the previous timestep of layer 2):
  gates1[64,512] = onehot(t) @ E1_s + sum_k h1T(t-1)_k @ Wh1_s[k]
  gates2[64,512] = ones1 @ b2_s + sum_k h1T(t-1)_k @ Wx2_s[k]
                                + sum_k h2T(t-2)_k @ Wh2_s[k]
  LSTM cell on stacked [128, 128] tiles -> h slice [128(batch x 2 layers),
  128(H-slice)]; PE-transpose -> payload [128(H-slice), h1T|h2T];
  AllGather payloads -> buf [128, 8*128]; out(t-2) = h2(t-2) @ W_out.
"""
import numpy as np

V, H, L, B, T = 128, 1024, 2, 64, 512
NCORES = 8
GS = 512          # per-core gate slice width
KT = H // 128     # 8 contraction tiles


def _build_nc():
    import concourse.bass as bass
    import concourse.mybir as mybir
    from concourse import bacc
    from concourse.tile import TileContext
    from concourse.masks import make_identity

    f32 = mybir.dt.float32
    AF = mybir.ActivationFunctionType
    OP = mybir.AluOpType

    nc = bacc.Bacc("TRN2", target_bir_lowering=False, name="charlstm",
                   num_devices=NCORES)

    d_wh1 = nc.dram_tensor("wh1", [KT, 128, GS], f32, kind="ExternalInput")
    d_wx2 = nc.dram_tensor("wx2", [KT, 128, GS], f32, kind="ExternalInput")
    d_wh2 = nc.dram_tensor("wh2", [KT, 128, GS], f32, kind="ExternalInput")
    d_e1 = nc.dram_tensor("e1", [128, GS], f32, kind="ExternalInput")
    d_b2 = nc.dram_tensor("b2", [1, GS], f32, kind="ExternalInput")
    d_wout = nc.dram_tensor("wout", [KT, 128, V], f32, kind="ExternalInput")
    d_oh = nc.dram_tensor("oh", [T, 128, B], f32, kind="ExternalInput")
    d_out = nc.dram_tensor("out", [T * B, V], f32, kind="ExternalOutput")

    ag_in = [nc.dram_tensor(f"ag_in{i}", [128, 128], f32) for i in range(2)]
    ag_out = [nc.dram_tensor(f"ag_out{i}", [NCORES * 128, 128], f32,
                             addr_space="Shared") for i in range(2)]
    rg = [list(range(NCORES))]

    with TileContext(nc) as tc:
        with tc.tile_pool(name="const", bufs=1) as cpool, \
             tc.tile_pool(name="oh", bufs=3) as ohpool, \
             tc.tile_pool(name="gps", bufs=2, space="PSUM") as gpspool, \
             tc.tile_pool(name="tps", bufs=2, space="PSUM") as tpspool, \
             tc.tile_pool(name="ops", bufs=2, space="PSUM") as opspool, \
             tc.tile_pool(name="work", bufs=2) as wpool:

            wh1 = cpool.tile([128, KT * GS], f32, tag="wh1")
            wx2 = cpool.tile([128, KT * GS], f32, tag="wx2")
            wh2 = cpool.tile([128, KT * GS], f32, tag="wh2")
            e1 = cpool.tile([128, GS], f32, tag="e1")
            b2 = cpool.tile([1, GS], f32, tag="b2")
            wout = cpool.tile([128, KT * V], f32, tag="wout")
            ident = cpool.tile([128, 128], f32, tag="ident")
            ones1 = cpool.tile([1, B], f32, tag="ones1")
            buf = []
            for i in range(2):
                bufi = cpool.tile([128, NCORES * 128], f32, tag=f"buf{i}",
                                  name=f"buf{i}")
                buf.append(bufi)

            for kt in range(KT):
                nc.sync.dma_start(wh1[:, kt * GS:(kt + 1) * GS], d_wh1[kt])
                nc.sync.dma_start(wx2[:, kt * GS:(kt + 1) * GS], d_wx2[kt])
                nc.sync.dma_start(wh2[:, kt * GS:(kt + 1) * GS], d_wh2[kt])
                nc.sync.dma_start(wout[:, kt * V:(kt + 1) * V], d_wout[kt])
            nc.sync.dma_start(e1[:], d_e1[:])
            nc.sync.dma_start(b2[:], d_b2[:])
            make_identity(nc, ident[:])
            nc.vector.memset(ones1[:], 1.0)
            nc.vector.memset(buf[0][:], 0.0)

            c_prev = wpool.tile([128, 128], f32, tag="c")
            nc.vector.memset(c_prev[:], 0.0)

            for tick in range(T + 2):
                par, nxt = tick % 2, (tick + 1) % 2
                l1 = tick < T          # layer-1 step `tick`
                l2 = 1 <= tick <= T    # layer-2 step `tick-1`
                do_out = 2 <= tick     # out step `tick-2`
                bsrc = buf[par]

                if l1 or l2:
                    g_ps = gpspool.tile([128, GS], f32, tag="g")
                if l1:
                    oh = ohpool.tile([128, B], f32, tag="oh")
                    nc.sync.dma_start(oh[:], d_oh[tick])
                    nc.tensor.matmul(g_ps[0:64, :], oh[:], e1[:],
                                     start=True, stop=False)
                    for kt in range(KT):
                        nc.tensor.matmul(
                            g_ps[0:64, :],
                            bsrc[:, kt * 128:kt * 128 + 64],
                            wh1[:, kt * GS:(kt + 1) * GS],
                            start=False, stop=(kt == KT - 1))
                if l2:
                    nc.tensor.matmul(g_ps[64:128, :], ones1[0:1, :],
                                     b2[0:1, :], start=True, stop=False,
                                     tile_position=(0, 64))
                    for kt in range(KT):
                        nc.tensor.matmul(
                            g_ps[64:128, :],
                            bsrc[:, kt * 128:kt * 128 + 64],
                            wx2[:, kt * GS:(kt + 1) * GS],
                            start=False, stop=False, tile_position=(0, 64))
                    for kt in range(KT):
                        nc.tensor.matmul(
                            g_ps[64:128, :],
                            bsrc[:, kt * 128 + 64:kt * 128 + 128],
                            wh2[:, kt * GS:(kt + 1) * GS],
                            start=False, stop=(kt == KT - 1),
                            tile_position=(0, 64))

                if l1 or l2:
                    lo, hi = (0, 128) if (l1 and l2) else ((0, 64) if l1 else (64, 128))
                    ifo = wpool.tile([128, 384], f32, tag="ifo")
                    gg = wpool.tile([128, 128], f32, tag="gg")
                    nc.scalar.activation(ifo[lo:hi, :], g_ps[lo:hi, 0:384],
                                         AF.Sigmoid)
                    nc.scalar.activation(gg[lo:hi, :], g_ps[lo:hi, 384:512],
                                         AF.Tanh)
                    t1 = wpool.tile([128, 128], f32, tag="t1")
                    t2 = wpool.tile([128, 128], f32, tag="t2")
                    c_new = wpool.tile([128, 128], f32, tag="c")
                    nc.vector.tensor_mul(t1[lo:hi, :], ifo[lo:hi, 0:128],
                                         gg[lo:hi, :])
                    nc.vector.tensor_mul(t2[lo:hi, :], ifo[lo:hi, 128:256],
                                         c_prev[lo:hi, :])
                    nc.vector.tensor_add(c_new[lo:hi, :], t1[lo:hi, :],
                                         t2[lo:hi, :])
                    if tick == 0:
                        nc.vector.memset(c_new[64:128, :], 0.0)
                    tch = wpool.tile([128, 128], f32, tag="tch")
                    nc.scalar.activation(tch[lo:hi, :], c_new[lo:hi, :],
                                         AF.Tanh)
                    h_sb = wpool.tile([128, 128], f32, tag="h")
                    nc.vector.tensor_mul(h_sb[lo:hi, :], ifo[lo:hi, 256:384],
                                         tch[lo:hi, :])
                    if tick == 0:
                        nc.vector.memset(h_sb[64:128, :], 0.0)
                    if tick == T:
                        nc.vector.memset(h_sb[0:64, :], 0.0)
                    c_prev = c_new

                if tick <= T:
                    pT = tpspool.tile([128, 128], f32, tag="pT")
                    nc.tensor.transpose(pT[:], h_sb[:], ident[:])
                    pay = wpool.tile([128, 128], f32, tag="pay")
                    nc.vector.tensor_copy(pay[:], pT[:])
                    nc.sync.dma_start(ag_in[par][:], pay[:])
                    nc.gpsimd.collective_compute(
                        "AllGather", mybir.AluOpType.bypass,
                        ins=[ag_in[par][:]], outs=[ag_out[par][:]],
                        replica_groups=rg)
                    nc.sync.dma_start(
                        buf[nxt][:].rearrange("p (r c) -> p r c", r=NCORES),
                        ag_out[par][:].rearrange("(r p) c -> p r c", r=NCORES))

                if do_out:
                    o_ps = opspool.tile([64, V], f32, tag="o")
                    for kt in range(KT):
                        nc.tensor.matmul(
                            o_ps[:],
                            bsrc[:, kt * 128 + 64:kt * 128 + 128],
                            wout[:, kt * V:(kt + 1) * V],
                            start=(kt == 0), stop=(kt == KT - 1))
                    o_sb = wpool.tile([64, V], f32, tag="osb")
                    nc.vector.tensor_copy(o_sb[:], o_ps[:])
                    nc.sync.dma_start(d_out[(tick - 2) * B:(tick - 1) * B, :],
                                      o_sb[:])

    nc.compile()
    return nc


def _host_prep(idx, embed, Wx, Wh, b, W_out):
    """Slice/pack weights per core. Gate column order [i|f|o|g]."""
    idx = np.asarray(idx)
    embed = np.asarray(embed, np.float32)
    Wx = np.asarray(Wx, np.float32)
    Wh = np.asarray(Wh, np.float32)
    b = np.asarray(b, np.float32)
    W_out = np.asarray(W_out, np.float32)

    E1 = embed @ Wx[0] + b[0]          # (V, 4H)
    onehot = (idx.T[:, None, :] == np.arange(V, dtype=idx.dtype)[None, :, None])
    oh = np.ascontiguousarray(onehot.astype(np.float32))      # (T, V, B)

    def cols(j):
        return np.concatenate([np.arange(g * H + j * 128, g * H + (j + 1) * 128)
                               for g in (0, 1, 3, 2)])  # i, f, o, g

    in_maps = []
    wout_t = np.ascontiguousarray(W_out.reshape(KT, 128, V))
    for j in range(NCORES):
        cj = cols(j)
        in_maps.append({
            "wh1": np.ascontiguousarray(Wh[0][:, cj].reshape(KT, 128, GS)),
            "wx2": np.ascontiguousarray(Wx[1][:, cj].reshape(KT, 128, GS)),
            "wh2": np.ascontiguousarray(Wh[1][:, cj].reshape(KT, 128, GS)),
            "e1": np.ascontiguousarray(E1[:, cj]),
            "b2": np.ascontiguousarray(b[1][cj][None, :]),
            "wout": wout_t,
            "oh": oh,
        })
    return in_maps


_NC_CACHE = {}


def kernel(idx, embed, Wx, Wh, b, W_out):
    from concourse.bass_interp import get_hw_module
    from concourse.bass_utils import run_bass_kernel_spmd

    if "nc" not in _NC_CACHE:
        nc = _build_nc()
        nc.m = get_hw_module(nc.m)
        _NC_CACHE["nc"] = nc
    nc = _NC_CACHE["nc"]

    in_maps = _host_prep(idx, embed, Wx, Wh, b, W_out)
    res = run_bass_kernel_spmd(nc, in_maps, core_ids=list(range(NCORES)))
    _NC_CACHE["last_results"] = res
    out = res.results[0]["out"]          # (T*B, V), t-major
    return np.ascontiguousarray(
        out.reshape(T, B, V).transpose(1, 0, 2)).astype(np.float32)
